# revision 26
# baseline (speedup 1.0000x reference)
"""Trainium2 Bass kernel for nn_BDHLayer (sparse attention / BDH layer).

Sharding: 16 heads across 8 cores (2 heads per core, tensor parallel).
Decoder partial sums are combined with an on-chip ReduceScatter (bf16);
each core then applies the final layernorm+residual+rmsnorm to its T/8
slice.

All matmuls run in bf16 (fp32 PSUM accumulation). Host pre-transposes
weights/activations so every contraction dim lands on SBUF partitions.
The middle layernorm is applied as a post-GEMM correction:
  sqrelu(LN(yKV) @ Wv^T) = relu(z - Wsum*mu)^2 * r^2,
  z = yKV @ Wv^T, Wsum = sum_d Wv, r^2 = 1/(var+eps).

v2 restructure vs v1:
- enc loop tb-outer with per-(chunk, tb) segmented RoPE so scores start
  right after enc (no serial rope tail).
- yKV accumulation is column-trimmed on the causal diagonal like scores.
- LN stat rows broadcast via gpsimd partition_broadcast (PE never waits).
- Gating g = relu(z - Wsum*mu)^2 split across Act (relu, square) and DVE
  (v, xyw) so neither stalls the z matmuls.
- h1 gating runs tb-outer and the decoder chunk + ReduceScatter + final
  norms for each t-block are interleaved right behind it, hiding the
  collective latency behind remaining tensor work.
- Weight/activation DMAs are split and ordered so the first dependent
  matmul can start within a few us of each phase boundary.
"""

import sys

sys.path.insert(0, '/opt/trn_rl_repo')

import numpy as np
import ml_dtypes

import concourse.bass as bass
import concourse.bacc as bacc
import concourse.mybir as mybir
from concourse import tile
from concourse import bass_utils
from concourse import bass_isa

BF = ml_dtypes.bfloat16
FP32 = np.float32

B, T, D = 1, 2048, 1024
NH = 16
N = 1024            # neurons per head
CS = 256            # rotary chunk size
BASE = 2.0 ** 16
SCALE_BASE = 512.0
LN_EPS = 1e-5
RMS_EPS = 1e-6

NCORES = 8
HPC = NH // NCORES  # heads per core = 2
TS = T // NCORES    # output rows per core = 256

NT = N // 128       # 8 n-tiles per head
DT = D // 128       # 8 d-tiles
TT = T // 128       # 16 t-tiles
TB = T // 512       # 4 t-blocks
DB = D // 512       # 2 d-blocks

dt = mybir.dt
Alu = mybir.AluOpType
Act = mybir.ActivationFunctionType


# ---------------------------------------------------------------- host tables

def _rope_tables():
    idx = np.arange(0, CS, 2, dtype=np.float64)
    inv_freq = 1.0 / (BASE ** (idx / CS))
    t = np.arange(T, dtype=np.float64)
    freqs = t[:, None] * inv_freq[None, :]              # (T, 128)
    scale_vec = (idx + 0.4 * CS) / (1.4 * CS)
    power = (t - T // 2) / SCALE_BASE
    scale = scale_vec[None, :] ** power[:, None]        # (T, 128)
    cos = (np.cos(freqs) * scale).astype(np.float32)
    sin = (np.sin(freqs) * scale).astype(np.float32)
    # transpose to (128, T): row = pair index within chunk, col = t
    return np.ascontiguousarray(cos.T), np.ascontiguousarray(sin.T)


def _masks():
    # scoresT tile layout: [u_p (128), t_f (512)]; diagonal block j keeps
    # strictly-causal u < t, i.e. 128*j + u_p < t_f.
    m = np.zeros((4, 128, 512), dtype=np.float32)
    up = np.arange(128)[:, None]
    tf = np.arange(512)[None, :]
    for j in range(4):
        m[j] = (128 * j + up < tf).astype(np.float32)
    return m


# ------------------------------------------------------------------- builder

def _emit(nc, tc, tens):
    x_bf, xT_bf, xs_f32 = tens['x_bf'], tens['xT_bf'], tens['xs_f32']
    wencT, wencvT, wdecT = tens['wencT'], tens['wencvT'], tens['wdecT']
    wsumT, cosT_d, sinT_d, masks_d = (tens['wsumT'], tens['cosT'],
                                      tens['sinT'], tens['masks'])
    out_d, xy_d = tens['out'], tens['xy_d']
    bounce_in, bounce_out = tens['bounce_in'], tens['bounce_out']

    f32, bf16 = dt.float32, dt.bfloat16

    from contextlib import ExitStack
    with ExitStack() as ctx:
        p_const = ctx.enter_context(
            tc.tile_pool(name="const", bufs=1, side="right"))
        p_psum = ctx.enter_context(
            tc.tile_pool(name="psum", bufs=6, space="PSUM"))
        p_psum_v = ctx.enter_context(
            tc.tile_pool(name="psumv", bufs=2, space="PSUM"))

        # ---- constants; their DMAs are issued inside h0's enc scope so the
        # startup HBM bandwidth goes to the first-needed weights first
        cos_sb = p_const.tile([128, T], bf16, tag="cos")
        sin_sb = p_const.tile([128, T], bf16, tag="sin")
        mask_sb = p_const.tile([128, 4 * 512], bf16, tag="masks")
        wsum_sb = p_const.tile([128, HPC * NT], f32, tag="wsum")
        ones_bf = p_const.tile([128, 1], bf16, tag="ones_bf")
        nc.vector.memset(ones_bf[:], 1.0)

        for h in range(HPC):
            with ExitStack() as hctx:
                p_head = hctx.enter_context(
                    tc.tile_pool(name=f"head{h}", bufs=1, side="right"))
                qsq = p_head.tile([128, NT * T], bf16, tag="qsq")
                ykv = p_head.tile([128, DT * T], bf16, tag="ykv")
                mu_b = p_head.tile([128, T], bf16, tag="mu_b")
                r2_b = p_head.tile([128, T], bf16, tag="r2_b")
                p_wv = hctx.enter_context(
                    tc.tile_pool(name=f"wv{h}", bufs=1, side="left"))
                wv_sb = p_wv.tile([128, DT * N], bf16, tag="wv")

                with ExitStack() as mctx:
                    p_mid = mctx.enter_context(
                        tc.tile_pool(name=f"mid{h}", bufs=1, side="right"))
                    qtr = p_mid.tile([128, NT * T], bf16, tag="qtr")
                    x_sb = p_mid.tile([128, TT * D], bf16, tag="x")

                    # =========================== ENC + ROPE ================
                    with ExitStack() as ectx:
                        p_enc = ectx.enter_context(
                            tc.tile_pool(name=f"enc{h}", bufs=1))
                        p_scr = ectx.enter_context(
                            tc.tile_pool(name=f"escr{h}", bufs=3))
                        p_rt = ectx.enter_context(
                            tc.tile_pool(name=f"rt{h}", bufs=4))

                        p_xf = ectx.enter_context(
                            tc.tile_pool(name=f"xf{h}", bufs=2))
                        wenc_sb = p_enc.tile([128, DT * N], bf16, tag="wenc")
                        # first-needed first: wenc cols 0:512 + xfull tb0,
                        # then the rest; x_sb (for yKV) trails behind.
                        # h0 startup splits across sync+gpsimd; h1's loads
                        # stay off gpsimd (it is busy storing h0's xy).
                        eng2 = nc.gpsimd if h == 0 else nc.sync

                        def load_xf(tb):
                            t = p_xf.tile([128, DT * 512], bf16, tag="xf")
                            for dtt in range(DT):
                                eng = eng2 if (tb + dtt) % 2 == 0 else nc.sync
                                eng.dma_start(
                                    t[:, dtt * 512:(dtt + 1) * 512],
                                    xT_bf[dtt * 128:(dtt + 1) * 128,
                                          tb * 512:(tb + 1) * 512])
                            return t

                        for dtt in range(DT):
                            nc.sync.dma_start(
                                wenc_sb[:, dtt * N:dtt * N + 512],
                                wencT[h, dtt * 128:(dtt + 1) * 128, 0:512])
                        if h == 0:
                            xf_t = p_xf.tile([128, DT * 512], bf16, tag="xf")
                            for dtt in range(DT):
                                nc.gpsimd.dma_start(
                                    xf_t[:, dtt * 512:(dtt + 1) * 512],
                                    xT_bf[dtt * 128:(dtt + 1) * 128, 0:512])
                        else:
                            xf_t = load_xf(0)
                        for dtt in range(DT):
                            nc.sync.dma_start(
                                wenc_sb[:, dtt * N + 512:(dtt + 1) * N],
                                wencT[h, dtt * 128:(dtt + 1) * 128, 512:N])
                        if h == 0:
                            # constants ride behind the critical enc loads
                            nc.gpsimd.dma_start(cos_sb[:], cosT_d[:])
                            nc.gpsimd.dma_start(sin_sb[:], sinT_d[:])
                        xf_next = load_xf(1)
                        for tt in range(TT):
                            eng = nc.sync if tt % 2 == 0 else eng2
                            eng.dma_start(x_sb[:, tt * D:(tt + 1) * D],
                                          x_bf[tt * 128:(tt + 1) * 128, :])
                        if h == 0:
                            for j in range(4):
                                nc.gpsimd.dma_start(
                                    mask_sb[:, j * 512:(j + 1) * 512],
                                    masks_d[j, :, :])
                            for hh in range(HPC):
                                nc.gpsimd.dma_start(
                                    wsum_sb[:, hh * NT:(hh + 1) * NT],
                                    wsumT[hh, :, :])

                        for tb in range(TB):
                            tsl = slice(tb * 512, (tb + 1) * 512)
                            xfull = xf_t
                            for nt in range(NT):
                                ps = p_psum.tile([128, 512], f32, tag="mm")
                                for dtt in range(DT):
                                    nc.tensor.matmul(
                                        ps[:],
                                        wenc_sb[:, dtt * N + nt * 128:
                                                dtt * N + nt * 128 + 128],
                                        xfull[:, dtt * 512:(dtt + 1) * 512],
                                        start=(dtt == 0), stop=(dtt == DT - 1))
                                relu_t = p_scr.tile([128, 512], f32,
                                                    tag="relu")
                                nc.scalar.activation(relu_t[:], ps[:],
                                                     Act.Relu)
                                nc.vector.tensor_mul(
                                    qsq[:, nt * T + tb * 512:
                                        nt * T + tb * 512 + 512],
                                    relu_t[:], relu_t[:])
                                if nt % 2 == 1:
                                    # rope this (chunk pair, tb) segment now
                                    c = nt // 2
                                    a = qsq[:, (2 * c) * T + tb * 512:
                                            (2 * c) * T + (tb + 1) * 512]
                                    b = qsq[:, (2 * c + 1) * T + tb * 512:
                                            (2 * c + 1) * T + (tb + 1) * 512]
                                    t1 = p_rt.tile([128, 512], bf16, tag="rt")
                                    t2 = p_rt.tile([128, 512], bf16, tag="rt")
                                    nc.vector.tensor_mul(t1[:], a,
                                                         cos_sb[:, tsl])
                                    nc.vector.tensor_mul(t2[:], b,
                                                         sin_sb[:, tsl])
                                    nc.vector.tensor_sub(
                                        qtr[:, (2 * c) * T + tb * 512:
                                            (2 * c) * T + (tb + 1) * 512],
                                        t1[:], t2[:])
                                    t3 = p_rt.tile([128, 512], bf16, tag="rt")
                                    t4 = p_rt.tile([128, 512], bf16, tag="rt")
                                    nc.vector.tensor_mul(t3[:], b,
                                                         cos_sb[:, tsl])
                                    nc.vector.tensor_mul(t4[:], a,
                                                         sin_sb[:, tsl])
                                    nc.vector.tensor_add(
                                        qtr[:, (2 * c + 1) * T + tb * 512:
                                            (2 * c + 1) * T + (tb + 1) * 512],
                                        t3[:], t4[:])
                            xf_t = xf_next
                            if tb < TB - 2:
                                xf_next = load_xf(tb + 2)

                    # ======================= SCORES + yKV ===================
                    with ExitStack() as sctx:
                        p_sct = sctx.enter_context(
                            tc.tile_pool(name=f"sct{h}", bufs=2))
                        p_sq = sctx.enter_context(
                            tc.tile_pool(name=f"sq{h}", bufs=1))
                        p_row = sctx.enter_context(
                            tc.tile_pool(name=f"row{h}", bufs=1))

                        # wv lands during the scores phase so the gating z
                        # matmuls never wait on it
                        for dtt in range(DT):
                            nc.sync.dma_start(
                                wv_sb[:, dtt * N:(dtt + 1) * N],
                                wencvT[h, dtt * 128:(dtt + 1) * 128, :])

                        for tb in range(TB):
                            ub_max = 4 * tb + 4
                            sct = p_sct.tile([128, 16 * 512], bf16, tag="sct")
                            for ub in range(ub_max):
                                j = ub - 4 * tb
                                off = 128 * j if j > 0 else 0
                                w = 512 - off
                                ps = p_psum.tile([128, 512], f32, tag="mm")
                                for nt in range(NT):
                                    nc.tensor.matmul(
                                        ps[:, :w],
                                        qtr[:, nt * T + ub * 128:
                                            nt * T + ub * 128 + 128],
                                        qtr[:, nt * T + tb * 512 + off:
                                            nt * T + (tb + 1) * 512],
                                        start=(nt == 0), stop=(nt == NT - 1))
                                base = ub * 512
                                if j >= 0:
                                    nc.vector.tensor_mul(
                                        sct[:, base + off:base + 512],
                                        ps[:, :w],
                                        mask_sb[:, j * 512 + off:
                                                (j + 1) * 512])
                                else:
                                    nc.scalar.copy(sct[:, base:base + 512],
                                                   ps[:])

                            sq_half = p_sq.tile([128, 4 * 512], bf16,
                                                tag="sq")
                            ssq_ps = p_psum_v.tile([1, 512], f32, tag="st")
                            for dtt in range(DT):
                                ps2 = p_psum.tile([128, 512], f32, tag="mm")
                                for ub in range(ub_max):
                                    j = ub - 4 * tb
                                    off = 128 * j if j > 0 else 0
                                    nc.tensor.matmul(
                                        ps2[:, off:],
                                        x_sb[:, ub * D + dtt * 128:
                                             ub * D + dtt * 128 + 128],
                                        sct[:, ub * 512 + off:
                                            (ub + 1) * 512],
                                        start=(ub == 0),
                                        stop=(ub == ub_max - 1))
                                nc.scalar.copy(
                                    ykv[:, dtt * T + tb * 512:
                                        dtt * T + tb * 512 + 512], ps2[:])
                                nc.scalar.square(
                                    sq_half[:, (dtt % 4) * 512:
                                            (dtt % 4 + 1) * 512],
                                    ps2[:])
                                if dtt % 4 == 3:
                                    for q4 in range(4):
                                        nc.tensor.matmul(
                                            ssq_ps[:], ones_bf[:],
                                            sq_half[:, q4 * 512:
                                                    (q4 + 1) * 512],
                                            start=(dtt == 3 and q4 == 0),
                                            stop=(dtt == DT - 1 and q4 == 3))

                            mean_ps = p_psum_v.tile([1, 512], f32, tag="st")
                            for dtt in range(DT):
                                nc.tensor.matmul(
                                    mean_ps[:], ones_bf[:],
                                    ykv[:, dtt * T + tb * 512:
                                        dtt * T + tb * 512 + 512],
                                    start=(dtt == 0), stop=(dtt == DT - 1))
                            sl = slice(tb * 512, (tb + 1) * 512)
                            mu_r = p_row.tile([1, 512], bf16, tag="mu_r")
                            ssq_r = p_row.tile([1, 512], f32, tag="ssq_r")
                            musq_r = p_row.tile([1, 512], bf16, tag="musq_r")
                            r2_r = p_row.tile([1, 512], bf16, tag="r2_r")
                            nc.scalar.mul(mu_r[:], mean_ps[:], 1.0 / D)
                            nc.scalar.mul(ssq_r[:], ssq_ps[:], 1.0 / D)
                            nc.vector.tensor_mul(musq_r[:], mu_r[:], mu_r[:])
                            nc.vector.tensor_sub(ssq_r[:], ssq_r[:],
                                                 musq_r[:])
                            nc.vector.tensor_scalar_add(
                                ssq_r[:], ssq_r[:], LN_EPS)
                            nc.vector.reciprocal(ssq_r[:], ssq_r[:])
                            nc.vector.tensor_copy(r2_r[:], ssq_r[:])
                            # broadcast stat rows to all partitions (gpsimd,
                            # keeps the PE out of the dependency chain)
                            nc.gpsimd.partition_broadcast(
                                mu_b[:, sl], mu_r[:], channels=128)
                            nc.gpsimd.partition_broadcast(
                                r2_b[:, sl], r2_r[:], channels=128)

                # ================== Z / GATING (+ DECODER on h1) ===========
                with ExitStack() as gctx:
                    p_zs = gctx.enter_context(
                        tc.tile_pool(name=f"zs{h}", bufs=3, side="left"))
                    p_zq = gctx.enter_context(
                        tc.tile_pool(name=f"zq{h}", bufs=3, side="left"))

                    if h == 0:
                        p_xyw = gctx.enter_context(
                            tc.tile_pool(name="xyw0", bufs=3, side="left"))
                    else:
                        p_wd = gctx.enter_context(
                            tc.tile_pool(name="wd", bufs=1, side="left"))
                        p_xy1 = gctx.enter_context(
                            tc.tile_pool(name="xy1", bufs=2, side="left"))
                        p_xy0 = gctx.enter_context(
                            tc.tile_pool(name="xy0", bufs=2, side="left"))
                        p_ym = gctx.enter_context(
                            tc.tile_pool(name="ym", bufs=3, side="left"))
                        p_fin = gctx.enter_context(
                            tc.tile_pool(name="fin", bufs=1, side="left"))
                        wd_sb = p_wd.tile([128, HPC * NT * D], bf16, tag="wd")
                        for r in range(HPC * NT):
                            eng = nc.sync if r % 2 == 1 else nc.gpsimd
                            eng.dma_start(wd_sb[:, r * D:(r + 1) * D],
                                          wdecT[r * 128:(r + 1) * 128, :])

                        def fetch_xy0(tb):
                            t = p_xy0.tile([128, NT * 512], bf16, tag="xy0")
                            for nt in range(NT):
                                nc.scalar.dma_start(
                                    t[:, nt * 512:(nt + 1) * 512],
                                    xy_d[nt, :, tb * 512:(tb + 1) * 512])
                            return t
                        xy0_t = fetch_xy0(0)
                        xy0_next = fetch_xy0(1)

                    for tb in range(TB):
                        sl = slice(tb * 512, (tb + 1) * 512)
                        if h == 1:
                            xy1 = p_xy1.tile([128, NT * 512], bf16, tag="xy1")
                        for nt in range(NT):
                            q_t = p_zq.tile([128, 512], bf16, tag="q")
                            nc.vector.tensor_mul(
                                q_t[:], qsq[:, nt * T + tb * 512:
                                            nt * T + (tb + 1) * 512],
                                r2_b[:, sl])
                            ps3 = p_psum.tile([128, 512], f32, tag="mm")
                            for dtt in range(DT):
                                nc.tensor.matmul(
                                    ps3[:],
                                    wv_sb[:, dtt * N + nt * 128:
                                          dtt * N + nt * 128 + 128],
                                    ykv[:, dtt * T + tb * 512:
                                        dtt * T + tb * 512 + 512],
                                    start=(dtt == 0), stop=(dtt == DT - 1))
                            # v = Wsum[n]*mu[t] - z ; g = relu(-v)^2
                            v_t = p_zs.tile([128, 512], f32, tag="v")
                            nc.vector.scalar_tensor_tensor(
                                v_t[:], mu_b[:, sl],
                                wsum_sb[:, h * NT + nt:h * NT + nt + 1],
                                ps3[:], op0=Alu.mult, op1=Alu.subtract)
                            t1_t = p_zs.tile([128, 512], bf16, tag="t1")
                            nc.scalar.activation(t1_t[:], v_t[:], Act.Relu,
                                                 scale=-1.0)
                            g_t = p_zs.tile([128, 512], bf16, tag="g")
                            nc.scalar.square(g_t[:], t1_t[:])
                            if h == 0:
                                xyw = p_xyw.tile([128, 512], bf16, tag="xyw")
                                nc.vector.tensor_mul(xyw[:], g_t[:], q_t[:])
                                nc.gpsimd.dma_start(
                                    xy_d[nt, :, tb * 512:(tb + 1) * 512],
                                    xyw[:])
                            else:
                                nc.vector.tensor_mul(
                                    xy1[:, nt * 512:(nt + 1) * 512],
                                    g_t[:], q_t[:])

                        if h == 1:
                            # -------- decoder + RS + final norms; last
                            # t-block splits in two so the tail collective
                            # is half-size
                            subs = ([(4 * tb, 4)] if tb < TB - 1
                                    else [(12, 2), (14, 1), (15, 1)])
                            for tt0, ntt in subs:
                                for tt in range(tt0, tt0 + ntt):
                                    to = (tt - 4 * tb) * 128
                                    for db in range(DB):
                                        ps4 = p_psum.tile([128, 512], f32,
                                                          tag="mm")
                                        idx = 0
                                        for hh in range(HPC):
                                            src = xy0_t if hh == 0 else xy1
                                            for nt in range(NT):
                                                nc.tensor.matmul(
                                                    ps4[:],
                                                    src[:, nt * 512 + to:
                                                        nt * 512 + to + 128],
                                                    wd_sb[:,
                                                          (hh * NT + nt) * D +
                                                          db * 512:
                                                          (hh * NT + nt) * D +
                                                          db * 512 + 512],
                                                    start=(idx == 0),
                                                    stop=(idx ==
                                                          HPC * NT - 1))
                                                idx += 1
                                        ym_t = p_ym.tile([128, 512], bf16,
                                                         tag="ym")
                                        nc.scalar.copy(ym_t[:], ps4[:])
                                        nc.sync.dma_start(
                                            bounce_in[tt * 128:(tt + 1) * 128,
                                                      db * 512:
                                                      (db + 1) * 512],
                                            ym_t[:])
                                r0 = tt0 * 128
                                rows = ntt * 128
                                o0 = r0 // NCORES
                                P = rows // NCORES
                                nc.gpsimd.collective_compute(
                                    "ReduceScatter", Alu.add,
                                    replica_groups=[list(range(NCORES))],
                                    ins=[bounce_in[r0:r0 + rows, :].opt()],
                                    outs=[bounce_out[o0:o0 + P, :].opt()])

                                # final norms for this chunk's P rows
                                PO = o0
                                yt = p_fin.tile([P, D], bf16, tag="yt")
                                nc.sync.dma_start(yt[:],
                                                  bounce_out[PO:PO + P, :])
                                xt = p_fin.tile([P, D], f32, tag="xt")
                                nc.sync.dma_start(xt[:],
                                                  xs_f32[PO:PO + P, :])

                                mu_c = p_fin.tile([P, 1], f32, tag="mu_c")
                                nc.vector.tensor_reduce(mu_c[:], yt[:],
                                                        mybir.AxisListType.X,
                                                        Alu.add)
                                nc.scalar.mul(mu_c[:], mu_c[:], 1.0 / D)
                                sq_t = p_fin.tile([P, D], f32, tag="sq_t")
                                ssq_c = p_fin.tile([P, 1], f32, tag="ssq_c")
                                nc.vector.tensor_mul(sq_t[:], yt[:], yt[:])
                                nc.vector.tensor_reduce(ssq_c[:], sq_t[:],
                                                        mybir.AxisListType.X,
                                                        Alu.add)
                                nc.scalar.mul(ssq_c[:], ssq_c[:], 1.0 / D)
                                musq_c = p_fin.tile([P, 1], f32,
                                                    tag="musq_c")
                                nc.vector.tensor_mul(musq_c[:], mu_c[:],
                                                     mu_c[:])
                                nc.vector.tensor_sub(ssq_c[:], ssq_c[:],
                                                     musq_c[:])
                                nc.vector.tensor_scalar_add(ssq_c[:],
                                                            ssq_c[:],
                                                            LN_EPS)
                                r_c = p_fin.tile([P, 1], f32, tag="r_c")
                                nc.vector.reciprocal(r_c[:], ssq_c[:])
                                nc.scalar.sqrt(r_c[:], r_c[:])

                                zt = p_fin.tile([P, D], f32, tag="zt")
                                nc.vector.tensor_scalar(zt[:], yt[:],
                                                        mu_c[:], r_c[:],
                                                        op0=Alu.subtract,
                                                        op1=Alu.mult)
                                nc.vector.tensor_add(zt[:], zt[:], xt[:])

                                nc.vector.tensor_mul(sq_t[:], zt[:], zt[:])
                                rr_c = p_fin.tile([P, 1], f32, tag="rr_c")
                                nc.vector.tensor_reduce(rr_c[:], sq_t[:],
                                                        mybir.AxisListType.X,
                                                        Alu.add)
                                nc.scalar.mul(rr_c[:], rr_c[:], 1.0 / D)
                                nc.vector.tensor_scalar_add(rr_c[:], rr_c[:],
                                                            RMS_EPS)
                                nc.vector.reciprocal(rr_c[:], rr_c[:])
                                nc.scalar.sqrt(rr_c[:], rr_c[:])

                                ot = p_fin.tile([P, D], f32, tag="ot")
                                nc.vector.tensor_scalar_mul(ot[:], zt[:],
                                                            rr_c[:])
                                nc.sync.dma_start(out_d[PO:PO + P, :], ot[:])
                            xy0_t = xy0_next
                            if tb < TB - 2:
                                xy0_next = fetch_xy0(tb + 2)


def build(debug=False):
    nc = bacc.Bacc("TRN2", target_bir_lowering=False, debug=False,
                   num_devices=NCORES)
    f32, bf16 = dt.float32, dt.bfloat16
    tens = {
        'x_bf': nc.dram_tensor("x_bf", [T, D], bf16, kind="ExternalInput"),
        'xT_bf': nc.dram_tensor("xT_bf", [D, T], bf16, kind="ExternalInput"),
        'xs_f32': nc.dram_tensor("xs_f32", [TS, D], f32,
                                 kind="ExternalInput"),
        'wencT': nc.dram_tensor("wencT", [HPC, D, N], bf16,
                                kind="ExternalInput"),
        'wencvT': nc.dram_tensor("wencvT", [HPC, D, N], bf16,
                                 kind="ExternalInput"),
        'wdecT': nc.dram_tensor("wdecT", [HPC * N, D], bf16,
                                kind="ExternalInput"),
        'wsumT': nc.dram_tensor("wsumT", [HPC, 128, NT], f32,
                                kind="ExternalInput"),
        'cosT': nc.dram_tensor("cosT", [128, T], bf16, kind="ExternalInput"),
        'sinT': nc.dram_tensor("sinT", [128, T], bf16, kind="ExternalInput"),
        'masks': nc.dram_tensor("masks", [4, 128, 512], bf16,
                                kind="ExternalInput"),
        'out': nc.dram_tensor("out", [TS, D], f32, kind="ExternalOutput"),
        'xy_d': nc.dram_tensor("xy_d", [NT, 128, T], bf16, kind="Internal"),
        'bounce_in': nc.dram_tensor("bounce_in", [T, D], bf16,
                                    kind="Internal"),
        'bounce_out': nc.dram_tensor("bounce_out", [TS, D], bf16,
                                     kind="Internal"),
    }

    with tile.TileContext(nc) as tc:
        _emit(nc, tc, tens)
    nc.compile()
    return nc


def make_in_maps(x, W_enc, W_enc_v, W_dec):
    x2 = np.asarray(x, FP32).reshape(T, D)
    x_bf = x2.astype(BF)
    xT_bf = np.ascontiguousarray(x2.T).astype(BF)
    cosT, sinT = _rope_tables()
    cosT, sinT = cosT.astype(BF), sinT.astype(BF)
    masks = _masks().astype(BF)
    wsum = np.asarray(W_enc_v, FP32).sum(axis=2)          # (NH, N)

    in_maps = []
    for k in range(NCORES):
        h0 = HPC * k
        wencT = np.ascontiguousarray(
            np.asarray(W_enc[h0:h0 + HPC], FP32).transpose(0, 2, 1)
        ).astype(BF)
        wencvT = np.ascontiguousarray(
            np.asarray(W_enc_v[h0:h0 + HPC], FP32).transpose(0, 2, 1)
        ).astype(BF)
        wdecT = np.ascontiguousarray(
            np.asarray(W_dec[:, h0 * N:(h0 + HPC) * N], FP32).T
        ).astype(BF)
        wsumT = np.ascontiguousarray(
            wsum[h0:h0 + HPC].reshape(HPC, NT, 128).transpose(0, 2, 1))
        in_maps.append({
            'x_bf': x_bf,
            'xT_bf': xT_bf,
            'xs_f32': np.ascontiguousarray(np.concatenate(
                [x2[tt0 * 128 + pp * k:tt0 * 128 + pp * k + pp]
                 for tt0, pp in ((0, 64), (4, 64), (8, 64),
                                 (12, 32), (14, 16), (15, 16))], axis=0)),
            'wencT': wencT,
            'wencvT': wencvT,
            'wdecT': wdecT,
            'wsumT': wsumT,
            'cosT': cosT,
            'sinT': sinT,
            'masks': masks,
        })
    return in_maps


_nc_cache = {}


def get_nc(debug=False):
    if debug not in _nc_cache:
        _nc_cache[debug] = build(debug=debug)
    return _nc_cache[debug]


def run(x, W_enc, W_enc_v, W_dec, debug=False, trace=False):
    nc = get_nc(debug=debug)
    in_maps = make_in_maps(x, W_enc, W_enc_v, W_dec)
    res = bass_utils.run_bass_kernel_spmd(
        nc, in_maps, core_ids=list(range(NCORES)), trace=trace)
    # chunked reduce-scatter: core c's piece i holds the c-th 1/8 of
    # chunk i's row range
    out = np.empty((T, D), np.float32)
    for c in range(NCORES):
        oc = res.results[c]['out']
        o = 0
        for tt0, pp in ((0, 64), (4, 64), (8, 64), (12, 32),
                        (14, 16), (15, 16)):
            g = tt0 * 128 + pp * c
            out[g:g + pp] = oc[o:o + pp]
            o += pp
    return out.reshape(B, T, D), res


def kernel(x, W_enc, W_enc_v, W_dec):
    out, _ = run(x, W_enc, W_enc_v, W_dec)
    return out.astype(np.float32)


# revision 29
# speedup vs baseline: 1.0159x; 1.0159x over previous
"""Trainium2 Bass kernel for nn_BDHLayer (sparse attention / BDH layer).

Sharding: 16 heads across 8 cores (2 heads per core, tensor parallel).
Decoder partial sums are combined with an on-chip ReduceScatter (bf16);
each core then applies the final layernorm+residual+rmsnorm to its T/8
slice.

All matmuls run in bf16 (fp32 PSUM accumulation). Host pre-transposes
weights/activations so every contraction dim lands on SBUF partitions.
The middle layernorm is applied as a post-GEMM correction:
  sqrelu(LN(yKV) @ Wv^T) = relu(z - Wsum*mu)^2 * r^2,
  z = yKV @ Wv^T, Wsum = sum_d Wv, r^2 = 1/(var+eps).

v2 restructure vs v1:
- enc loop tb-outer with per-(chunk, tb) segmented RoPE so scores start
  right after enc (no serial rope tail).
- yKV accumulation is column-trimmed on the causal diagonal like scores.
- LN stat rows broadcast via gpsimd partition_broadcast (PE never waits).
- Gating g = relu(z - Wsum*mu)^2 split across Act (relu, square) and DVE
  (v, xyw) so neither stalls the z matmuls.
- h1 gating runs tb-outer and the decoder chunk + ReduceScatter + final
  norms for each t-block are interleaved right behind it, hiding the
  collective latency behind remaining tensor work.
- Weight/activation DMAs are split and ordered so the first dependent
  matmul can start within a few us of each phase boundary.
"""

import sys

sys.path.insert(0, '/opt/trn_rl_repo')

import numpy as np
import ml_dtypes

import concourse.bass as bass
import concourse.bacc as bacc
import concourse.mybir as mybir
from concourse import tile
from concourse import bass_utils
from concourse import bass_isa

BF = ml_dtypes.bfloat16
FP32 = np.float32

B, T, D = 1, 2048, 1024
NH = 16
N = 1024            # neurons per head
CS = 256            # rotary chunk size
BASE = 2.0 ** 16
SCALE_BASE = 512.0
LN_EPS = 1e-5
RMS_EPS = 1e-6

NCORES = 8
HPC = NH // NCORES  # heads per core = 2
TS = T // NCORES    # output rows per core = 256

NT = N // 128       # 8 n-tiles per head
DT = D // 128       # 8 d-tiles
TT = T // 128       # 16 t-tiles
TB = T // 512       # 4 t-blocks
DB = D // 512       # 2 d-blocks

dt = mybir.dt
Alu = mybir.AluOpType
Act = mybir.ActivationFunctionType


# ---------------------------------------------------------------- host tables

def _rope_tables():
    idx = np.arange(0, CS, 2, dtype=np.float64)
    inv_freq = 1.0 / (BASE ** (idx / CS))
    t = np.arange(T, dtype=np.float64)
    freqs = t[:, None] * inv_freq[None, :]              # (T, 128)
    scale_vec = (idx + 0.4 * CS) / (1.4 * CS)
    power = (t - T // 2) / SCALE_BASE
    scale = scale_vec[None, :] ** power[:, None]        # (T, 128)
    cos = (np.cos(freqs) * scale).astype(np.float32)
    sin = (np.sin(freqs) * scale).astype(np.float32)
    # transpose to (128, T): row = pair index within chunk, col = t
    return np.ascontiguousarray(cos.T), np.ascontiguousarray(sin.T)


def _masks():
    # scoresT tile layout: [u_p (128), t_f (512)]; diagonal block j keeps
    # strictly-causal u < t, i.e. 128*j + u_p < t_f.
    m = np.zeros((4, 128, 512), dtype=np.float32)
    up = np.arange(128)[:, None]
    tf = np.arange(512)[None, :]
    for j in range(4):
        m[j] = (128 * j + up < tf).astype(np.float32)
    return m


# ------------------------------------------------------------------- builder

def _emit(nc, tc, tens):
    x_bf, xT_bf, xs_f32 = tens['x_bf'], tens['xT_bf'], tens['xs_f32']
    wencT, wencvT, wdecT = tens['wencT'], tens['wencvT'], tens['wdecT']
    wsumT, cosT_d, sinT_d, masks_d = (tens['wsumT'], tens['cosT'],
                                      tens['sinT'], tens['masks'])
    out_d, xy_d = tens['out'], tens['xy_d']
    bounce_in, bounce_out = tens['bounce_in'], tens['bounce_out']

    f32, bf16 = dt.float32, dt.bfloat16

    from contextlib import ExitStack
    with ExitStack() as ctx:
        p_const = ctx.enter_context(
            tc.tile_pool(name="const", bufs=1, side="right"))
        p_psum = ctx.enter_context(
            tc.tile_pool(name="psum", bufs=6, space="PSUM"))
        p_psum_v = ctx.enter_context(
            tc.tile_pool(name="psumv", bufs=2, space="PSUM"))

        # ---- constants; their DMAs are issued inside h0's enc scope so the
        # startup HBM bandwidth goes to the first-needed weights first
        cos_sb = p_const.tile([128, T], bf16, tag="cos")
        sin_sb = p_const.tile([128, T], bf16, tag="sin")
        mask_sb = p_const.tile([128, 4 * 512], bf16, tag="masks")
        wsum_sb = p_const.tile([128, HPC * NT], f32, tag="wsum")
        ones_bf = p_const.tile([128, 1], bf16, tag="ones_bf")
        nc.vector.memset(ones_bf[:], 1.0)

        for h in range(HPC):
            with ExitStack() as hctx:
                p_head = hctx.enter_context(
                    tc.tile_pool(name=f"head{h}", bufs=1, side="right"))
                qsq = p_head.tile([128, NT * T], bf16, tag="qsq")
                ykv = p_head.tile([128, DT * T], bf16, tag="ykv")
                mu_b = p_head.tile([128, T], bf16, tag="mu_b")
                r2_b = p_head.tile([128, T], bf16, tag="r2_b")
                p_wv = hctx.enter_context(
                    tc.tile_pool(name=f"wv{h}", bufs=1, side="left"))
                wv_sb = p_wv.tile([128, DT * N], bf16, tag="wv")

                with ExitStack() as mctx:
                    p_mid = mctx.enter_context(
                        tc.tile_pool(name=f"mid{h}", bufs=1, side="right"))
                    qtr = p_mid.tile([128, NT * T], bf16, tag="qtr")
                    x_sb = p_mid.tile([128, TT * D], bf16, tag="x")

                    # =========================== ENC + ROPE ================
                    with ExitStack() as ectx:
                        p_enc = ectx.enter_context(
                            tc.tile_pool(name=f"enc{h}", bufs=1))
                        p_scr = ectx.enter_context(
                            tc.tile_pool(name=f"escr{h}", bufs=3))
                        p_rt = ectx.enter_context(
                            tc.tile_pool(name=f"rt{h}", bufs=4))

                        p_xf = ectx.enter_context(
                            tc.tile_pool(name=f"xf{h}", bufs=2))
                        wenc_sb = p_enc.tile([128, DT * N], bf16, tag="wenc")
                        # first-needed first: wenc cols 0:512 + xfull tb0,
                        # then the rest; x_sb (for yKV) trails behind.
                        # h0 startup splits across sync+gpsimd; h1's loads
                        # stay off gpsimd (it is busy storing h0's xy).
                        eng2 = nc.gpsimd if h == 0 else nc.sync

                        def load_xf(tb):
                            t = p_xf.tile([128, DT * 512], bf16, tag="xf")
                            for dtt in range(DT):
                                eng = eng2 if (tb + dtt) % 2 == 0 else nc.sync
                                eng.dma_start(
                                    t[:, dtt * 512:(dtt + 1) * 512],
                                    xT_bf[dtt * 128:(dtt + 1) * 128,
                                          tb * 512:(tb + 1) * 512])
                            return t

                        for dtt in range(DT):
                            nc.sync.dma_start(
                                wenc_sb[:, dtt * N:dtt * N + 512],
                                wencT[h, dtt * 128:(dtt + 1) * 128, 0:512])
                        if h == 0:
                            xf_t = p_xf.tile([128, DT * 512], bf16, tag="xf")
                            for dtt in range(DT):
                                nc.gpsimd.dma_start(
                                    xf_t[:, dtt * 512:(dtt + 1) * 512],
                                    xT_bf[dtt * 128:(dtt + 1) * 128, 0:512])
                        else:
                            xf_t = load_xf(0)
                        for dtt in range(DT):
                            nc.sync.dma_start(
                                wenc_sb[:, dtt * N + 512:(dtt + 1) * N],
                                wencT[h, dtt * 128:(dtt + 1) * 128, 512:N])
                        if h == 0:
                            # constants ride behind the critical enc loads
                            nc.gpsimd.dma_start(cos_sb[:], cosT_d[:])
                            nc.gpsimd.dma_start(sin_sb[:], sinT_d[:])
                        xf_next = load_xf(1)
                        for tt in range(TT):
                            eng = nc.sync if tt % 2 == 0 else eng2
                            eng.dma_start(x_sb[:, tt * D:(tt + 1) * D],
                                          x_bf[tt * 128:(tt + 1) * 128, :])
                        if h == 0:
                            for j in range(4):
                                nc.gpsimd.dma_start(
                                    mask_sb[:, j * 512:(j + 1) * 512],
                                    masks_d[j, :, :])
                            for hh in range(HPC):
                                nc.gpsimd.dma_start(
                                    wsum_sb[:, hh * NT:(hh + 1) * NT],
                                    wsumT[hh, :, :])

                        for tb in range(TB):
                            tsl = slice(tb * 512, (tb + 1) * 512)
                            xfull = xf_t
                            for nt in range(NT):
                                ps = p_psum.tile([128, 512], f32, tag="mm")
                                for dtt in range(DT):
                                    nc.tensor.matmul(
                                        ps[:],
                                        wenc_sb[:, dtt * N + nt * 128:
                                                dtt * N + nt * 128 + 128],
                                        xfull[:, dtt * 512:(dtt + 1) * 512],
                                        start=(dtt == 0), stop=(dtt == DT - 1))
                                relu_t = p_scr.tile([128, 512], f32,
                                                    tag="relu")
                                nc.scalar.activation(relu_t[:], ps[:],
                                                     Act.Relu)
                                nc.vector.tensor_mul(
                                    qsq[:, nt * T + tb * 512:
                                        nt * T + tb * 512 + 512],
                                    relu_t[:], relu_t[:])
                                if nt % 2 == 1:
                                    # rope this (chunk pair, tb) segment now
                                    c = nt // 2
                                    a = qsq[:, (2 * c) * T + tb * 512:
                                            (2 * c) * T + (tb + 1) * 512]
                                    b = qsq[:, (2 * c + 1) * T + tb * 512:
                                            (2 * c + 1) * T + (tb + 1) * 512]
                                    t1 = p_rt.tile([128, 512], bf16, tag="rt")
                                    t2 = p_rt.tile([128, 512], bf16, tag="rt")
                                    nc.vector.tensor_mul(t1[:], a,
                                                         cos_sb[:, tsl])
                                    nc.vector.tensor_mul(t2[:], b,
                                                         sin_sb[:, tsl])
                                    nc.vector.tensor_sub(
                                        qtr[:, (2 * c) * T + tb * 512:
                                            (2 * c) * T + (tb + 1) * 512],
                                        t1[:], t2[:])
                                    t3 = p_rt.tile([128, 512], bf16, tag="rt")
                                    t4 = p_rt.tile([128, 512], bf16, tag="rt")
                                    nc.vector.tensor_mul(t3[:], b,
                                                         cos_sb[:, tsl])
                                    nc.vector.tensor_mul(t4[:], a,
                                                         sin_sb[:, tsl])
                                    nc.vector.tensor_add(
                                        qtr[:, (2 * c + 1) * T + tb * 512:
                                            (2 * c + 1) * T + (tb + 1) * 512],
                                        t3[:], t4[:])
                            xf_t = xf_next
                            if tb < TB - 2:
                                xf_next = load_xf(tb + 2)

                    # ======================= SCORES + yKV ===================
                    with ExitStack() as sctx:
                        p_sct = sctx.enter_context(
                            tc.tile_pool(name=f"sct{h}", bufs=2))
                        p_sq = sctx.enter_context(
                            tc.tile_pool(name=f"sq{h}", bufs=1))
                        p_row = sctx.enter_context(
                            tc.tile_pool(name=f"row{h}", bufs=1))

                        # wv lands during the scores phase so the gating z
                        # matmuls never wait on it
                        for dtt in range(DT):
                            nc.sync.dma_start(
                                wv_sb[:, dtt * N:(dtt + 1) * N],
                                wencvT[h, dtt * 128:(dtt + 1) * 128, :])

                        for tb in range(TB):
                            ub_max = 4 * tb + 4
                            sct = p_sct.tile([128, 16 * 512], bf16, tag="sct")
                            for ub in range(ub_max):
                                j = ub - 4 * tb
                                off = 128 * j if j > 0 else 0
                                w = 512 - off
                                ps = p_psum.tile([128, 512], f32, tag="mm")
                                for nt in range(NT):
                                    nc.tensor.matmul(
                                        ps[:, :w],
                                        qtr[:, nt * T + ub * 128:
                                            nt * T + ub * 128 + 128],
                                        qtr[:, nt * T + tb * 512 + off:
                                            nt * T + (tb + 1) * 512],
                                        start=(nt == 0), stop=(nt == NT - 1))
                                base = ub * 512
                                if j >= 0:
                                    nc.vector.tensor_mul(
                                        sct[:, base + off:base + 512],
                                        ps[:, :w],
                                        mask_sb[:, j * 512 + off:
                                                (j + 1) * 512])
                                else:
                                    nc.scalar.copy(sct[:, base:base + 512],
                                                   ps[:])

                            sq_half = p_sq.tile([128, 4 * 512], bf16,
                                                tag="sq")
                            ssq_ps = p_psum_v.tile([1, 512], f32, tag="st")
                            for dtt in range(DT):
                                ps2 = p_psum.tile([128, 512], f32, tag="mm")
                                for ub in range(ub_max):
                                    j = ub - 4 * tb
                                    off = 128 * j if j > 0 else 0
                                    nc.tensor.matmul(
                                        ps2[:, off:],
                                        x_sb[:, ub * D + dtt * 128:
                                             ub * D + dtt * 128 + 128],
                                        sct[:, ub * 512 + off:
                                            (ub + 1) * 512],
                                        start=(ub == 0),
                                        stop=(ub == ub_max - 1))
                                nc.scalar.copy(
                                    ykv[:, dtt * T + tb * 512:
                                        dtt * T + tb * 512 + 512], ps2[:])
                                nc.scalar.square(
                                    sq_half[:, (dtt % 4) * 512:
                                            (dtt % 4 + 1) * 512],
                                    ps2[:])
                                if dtt % 4 == 3:
                                    for q4 in range(4):
                                        nc.tensor.matmul(
                                            ssq_ps[:], ones_bf[:],
                                            sq_half[:, q4 * 512:
                                                    (q4 + 1) * 512],
                                            start=(dtt == 3 and q4 == 0),
                                            stop=(dtt == DT - 1 and q4 == 3))

                            mean_ps = p_psum_v.tile([1, 512], f32, tag="st")
                            for dtt in range(DT):
                                nc.tensor.matmul(
                                    mean_ps[:], ones_bf[:],
                                    ykv[:, dtt * T + tb * 512:
                                        dtt * T + tb * 512 + 512],
                                    start=(dtt == 0), stop=(dtt == DT - 1))
                            sl = slice(tb * 512, (tb + 1) * 512)
                            mu_r = p_row.tile([1, 512], bf16, tag="mu_r")
                            ssq_r = p_row.tile([1, 512], f32, tag="ssq_r")
                            musq_r = p_row.tile([1, 512], bf16, tag="musq_r")
                            r2_r = p_row.tile([1, 512], bf16, tag="r2_r")
                            nc.scalar.mul(mu_r[:], mean_ps[:], 1.0 / D)
                            nc.scalar.mul(ssq_r[:], ssq_ps[:], 1.0 / D)
                            nc.vector.tensor_mul(musq_r[:], mu_r[:], mu_r[:])
                            nc.vector.tensor_sub(ssq_r[:], ssq_r[:],
                                                 musq_r[:])
                            nc.vector.tensor_scalar_add(
                                ssq_r[:], ssq_r[:], LN_EPS)
                            nc.vector.reciprocal(ssq_r[:], ssq_r[:])
                            nc.vector.tensor_copy(r2_r[:], ssq_r[:])
                            # broadcast stat rows to all partitions (gpsimd,
                            # keeps the PE out of the dependency chain)
                            nc.gpsimd.partition_broadcast(
                                mu_b[:, sl], mu_r[:], channels=128)
                            nc.gpsimd.partition_broadcast(
                                r2_b[:, sl], r2_r[:], channels=128)

                # ================== Z / GATING (+ DECODER on h1) ===========
                with ExitStack() as gctx:
                    p_zs = gctx.enter_context(
                        tc.tile_pool(name=f"zs{h}", bufs=3, side="left"))
                    p_zq = gctx.enter_context(
                        tc.tile_pool(name=f"zq{h}", bufs=3, side="left"))

                    if h == 0:
                        p_xyw = gctx.enter_context(
                            tc.tile_pool(name="xyw0", bufs=3, side="left"))
                    else:
                        p_wd = gctx.enter_context(
                            tc.tile_pool(name="wd", bufs=1, side="left"))
                        p_xy1 = gctx.enter_context(
                            tc.tile_pool(name="xy1", bufs=2, side="left"))
                        p_xy0 = gctx.enter_context(
                            tc.tile_pool(name="xy0", bufs=2, side="left"))
                        p_ym = gctx.enter_context(
                            tc.tile_pool(name="ym", bufs=3, side="left"))
                        p_fin = gctx.enter_context(
                            tc.tile_pool(name="fin", bufs=1, side="left"))
                        wd_sb = p_wd.tile([128, HPC * NT * D], bf16, tag="wd")
                        for r in range(HPC * NT):
                            eng = nc.sync if r % 2 == 1 else nc.gpsimd
                            eng.dma_start(wd_sb[:, r * D:(r + 1) * D],
                                          wdecT[r * 128:(r + 1) * 128, :])

                        def fetch_xy0(tb):
                            t = p_xy0.tile([128, NT * 512], bf16, tag="xy0")
                            for nt in range(NT):
                                nc.scalar.dma_start(
                                    t[:, nt * 512:(nt + 1) * 512],
                                    xy_d[nt, :, tb * 512:(tb + 1) * 512])
                            return t
                        xy0_t = fetch_xy0(0)
                        xy0_next = fetch_xy0(1)

                    for tb in range(TB):
                        sl = slice(tb * 512, (tb + 1) * 512)
                        if h == 1:
                            xy1 = p_xy1.tile([128, NT * 512], bf16, tag="xy1")
                        for nt in range(NT):
                            q_t = p_zq.tile([128, 512], bf16, tag="q")
                            nc.vector.tensor_mul(
                                q_t[:], qsq[:, nt * T + tb * 512:
                                            nt * T + (tb + 1) * 512],
                                r2_b[:, sl])
                            ps3 = p_psum.tile([128, 512], f32, tag="mm")
                            for dtt in range(DT):
                                nc.tensor.matmul(
                                    ps3[:],
                                    wv_sb[:, dtt * N + nt * 128:
                                          dtt * N + nt * 128 + 128],
                                    ykv[:, dtt * T + tb * 512:
                                        dtt * T + tb * 512 + 512],
                                    start=(dtt == 0), stop=(dtt == DT - 1))
                            # v = Wsum[n]*mu[t] - z ; g = relu(-v)^2
                            v_t = p_zs.tile([128, 512], f32, tag="v")
                            nc.vector.scalar_tensor_tensor(
                                v_t[:], mu_b[:, sl],
                                wsum_sb[:, h * NT + nt:h * NT + nt + 1],
                                ps3[:], op0=Alu.mult, op1=Alu.subtract)
                            t1_t = p_zs.tile([128, 512], bf16, tag="t1")
                            nc.scalar.activation(t1_t[:], v_t[:], Act.Relu,
                                                 scale=-1.0)
                            g_t = p_zs.tile([128, 512], bf16, tag="g")
                            nc.scalar.square(g_t[:], t1_t[:])
                            if h == 0:
                                xyw = p_xyw.tile([128, 512], bf16, tag="xyw")
                                nc.vector.tensor_mul(xyw[:], g_t[:], q_t[:])
                                nc.gpsimd.dma_start(
                                    xy_d[nt, :, tb * 512:(tb + 1) * 512],
                                    xyw[:])
                            else:
                                nc.vector.tensor_mul(
                                    xy1[:, nt * 512:(nt + 1) * 512],
                                    g_t[:], q_t[:])

                        if h == 1:
                            # -------- decoder + RS + final norms; last
                            # t-block splits in two so the tail collective
                            # is half-size
                            subs = ([(4 * tb, 4)] if tb < TB - 1
                                    else [(12, 2), (14, 2)])
                            for tt0, ntt in subs:
                                for tt in range(tt0, tt0 + ntt):
                                    to = (tt - 4 * tb) * 128
                                    for db in range(DB):
                                        ps4 = p_psum.tile([128, 512], f32,
                                                          tag="mm")
                                        idx = 0
                                        for hh in range(HPC):
                                            src = xy0_t if hh == 0 else xy1
                                            for nt in range(NT):
                                                nc.tensor.matmul(
                                                    ps4[:],
                                                    src[:, nt * 512 + to:
                                                        nt * 512 + to + 128],
                                                    wd_sb[:,
                                                          (hh * NT + nt) * D +
                                                          db * 512:
                                                          (hh * NT + nt) * D +
                                                          db * 512 + 512],
                                                    start=(idx == 0),
                                                    stop=(idx ==
                                                          HPC * NT - 1))
                                                idx += 1
                                        ym_t = p_ym.tile([128, 512], bf16,
                                                         tag="ym")
                                        nc.scalar.copy(ym_t[:], ps4[:])
                                        nc.sync.dma_start(
                                            bounce_in[tt * 128:(tt + 1) * 128,
                                                      db * 512:
                                                      (db + 1) * 512],
                                            ym_t[:])
                                r0 = tt0 * 128
                                rows = ntt * 128
                                o0 = r0 // NCORES
                                P = rows // NCORES
                                nc.gpsimd.collective_compute(
                                    "ReduceScatter", Alu.add,
                                    replica_groups=[list(range(NCORES))],
                                    ins=[bounce_in[r0:r0 + rows, :].opt()],
                                    outs=[bounce_out[o0:o0 + P, :].opt()])

                                # final norms for this chunk's P rows
                                PO = o0
                                yt = p_fin.tile([P, D], bf16, tag="yt")
                                nc.sync.dma_start(yt[:],
                                                  bounce_out[PO:PO + P, :])
                                xt = p_fin.tile([P, D], f32, tag="xt")
                                nc.sync.dma_start(xt[:],
                                                  xs_f32[PO:PO + P, :])

                                mu_c = p_fin.tile([P, 1], f32, tag="mu_c")
                                nc.vector.tensor_reduce(mu_c[:], yt[:],
                                                        mybir.AxisListType.X,
                                                        Alu.add)
                                nc.scalar.mul(mu_c[:], mu_c[:], 1.0 / D)
                                sq_t = p_fin.tile([P, D], f32, tag="sq_t")
                                ssq_c = p_fin.tile([P, 1], f32, tag="ssq_c")
                                nc.vector.tensor_mul(sq_t[:], yt[:], yt[:])
                                nc.vector.tensor_reduce(ssq_c[:], sq_t[:],
                                                        mybir.AxisListType.X,
                                                        Alu.add)
                                nc.scalar.mul(ssq_c[:], ssq_c[:], 1.0 / D)
                                musq_c = p_fin.tile([P, 1], f32,
                                                    tag="musq_c")
                                nc.vector.tensor_mul(musq_c[:], mu_c[:],
                                                     mu_c[:])
                                nc.vector.tensor_sub(ssq_c[:], ssq_c[:],
                                                     musq_c[:])
                                nc.vector.tensor_scalar_add(ssq_c[:],
                                                            ssq_c[:],
                                                            LN_EPS)
                                r_c = p_fin.tile([P, 1], f32, tag="r_c")
                                nc.vector.reciprocal(r_c[:], ssq_c[:])
                                nc.scalar.sqrt(r_c[:], r_c[:])

                                zt = p_fin.tile([P, D], f32, tag="zt")
                                nc.vector.tensor_scalar(zt[:], yt[:],
                                                        mu_c[:], r_c[:],
                                                        op0=Alu.subtract,
                                                        op1=Alu.mult)
                                nc.vector.tensor_add(zt[:], zt[:], xt[:])

                                nc.vector.tensor_mul(sq_t[:], zt[:], zt[:])
                                rr_c = p_fin.tile([P, 1], f32, tag="rr_c")
                                nc.vector.tensor_reduce(rr_c[:], sq_t[:],
                                                        mybir.AxisListType.X,
                                                        Alu.add)
                                nc.scalar.mul(rr_c[:], rr_c[:], 1.0 / D)
                                nc.vector.tensor_scalar_add(rr_c[:], rr_c[:],
                                                            RMS_EPS)
                                nc.vector.reciprocal(rr_c[:], rr_c[:])
                                nc.scalar.sqrt(rr_c[:], rr_c[:])

                                ot = p_fin.tile([P, D], f32, tag="ot")
                                nc.vector.tensor_scalar_mul(ot[:], zt[:],
                                                            rr_c[:])
                                nc.sync.dma_start(out_d[PO:PO + P, :], ot[:])
                            xy0_t = xy0_next
                            if tb < TB - 2:
                                xy0_next = fetch_xy0(tb + 2)


def build(debug=False):
    nc = bacc.Bacc("TRN2", target_bir_lowering=False, debug=False,
                   num_devices=NCORES)
    f32, bf16 = dt.float32, dt.bfloat16
    tens = {
        'x_bf': nc.dram_tensor("x_bf", [T, D], bf16, kind="ExternalInput"),
        'xT_bf': nc.dram_tensor("xT_bf", [D, T], bf16, kind="ExternalInput"),
        'xs_f32': nc.dram_tensor("xs_f32", [TS, D], f32,
                                 kind="ExternalInput"),
        'wencT': nc.dram_tensor("wencT", [HPC, D, N], bf16,
                                kind="ExternalInput"),
        'wencvT': nc.dram_tensor("wencvT", [HPC, D, N], bf16,
                                 kind="ExternalInput"),
        'wdecT': nc.dram_tensor("wdecT", [HPC * N, D], bf16,
                                kind="ExternalInput"),
        'wsumT': nc.dram_tensor("wsumT", [HPC, 128, NT], f32,
                                kind="ExternalInput"),
        'cosT': nc.dram_tensor("cosT", [128, T], bf16, kind="ExternalInput"),
        'sinT': nc.dram_tensor("sinT", [128, T], bf16, kind="ExternalInput"),
        'masks': nc.dram_tensor("masks", [4, 128, 512], bf16,
                                kind="ExternalInput"),
        'out': nc.dram_tensor("out", [TS, D], f32, kind="ExternalOutput"),
        'xy_d': nc.dram_tensor("xy_d", [NT, 128, T], bf16, kind="Internal"),
        'bounce_in': nc.dram_tensor("bounce_in", [T, D], bf16,
                                    kind="Internal"),
        'bounce_out': nc.dram_tensor("bounce_out", [TS, D], bf16,
                                     kind="Internal"),
    }

    with tile.TileContext(nc) as tc:
        _emit(nc, tc, tens)
    nc.compile()
    return nc


def make_in_maps(x, W_enc, W_enc_v, W_dec):
    x2 = np.asarray(x, FP32).reshape(T, D)
    x_bf = x2.astype(BF)
    xT_bf = np.ascontiguousarray(x2.T).astype(BF)
    cosT, sinT = _rope_tables()
    cosT, sinT = cosT.astype(BF), sinT.astype(BF)
    masks = _masks().astype(BF)
    wsum = np.asarray(W_enc_v, FP32).sum(axis=2)          # (NH, N)

    in_maps = []
    for k in range(NCORES):
        h0 = HPC * k
        wencT = np.ascontiguousarray(
            np.asarray(W_enc[h0:h0 + HPC], FP32).transpose(0, 2, 1)
        ).astype(BF)
        wencvT = np.ascontiguousarray(
            np.asarray(W_enc_v[h0:h0 + HPC], FP32).transpose(0, 2, 1)
        ).astype(BF)
        wdecT = np.ascontiguousarray(
            np.asarray(W_dec[:, h0 * N:(h0 + HPC) * N], FP32).T
        ).astype(BF)
        wsumT = np.ascontiguousarray(
            wsum[h0:h0 + HPC].reshape(HPC, NT, 128).transpose(0, 2, 1))
        in_maps.append({
            'x_bf': x_bf,
            'xT_bf': xT_bf,
            'xs_f32': np.ascontiguousarray(np.concatenate(
                [x2[tt0 * 128 + pp * k:tt0 * 128 + pp * k + pp]
                 for tt0, pp in ((0, 64), (4, 64), (8, 64),
                                 (12, 32), (14, 32))], axis=0)),
            'wencT': wencT,
            'wencvT': wencvT,
            'wdecT': wdecT,
            'wsumT': wsumT,
            'cosT': cosT,
            'sinT': sinT,
            'masks': masks,
        })
    return in_maps


_nc_cache = {}


def get_nc(debug=False):
    if debug not in _nc_cache:
        _nc_cache[debug] = build(debug=debug)
    return _nc_cache[debug]


def run(x, W_enc, W_enc_v, W_dec, debug=False, trace=False):
    nc = get_nc(debug=debug)
    in_maps = make_in_maps(x, W_enc, W_enc_v, W_dec)
    res = bass_utils.run_bass_kernel_spmd(
        nc, in_maps, core_ids=list(range(NCORES)), trace=trace)
    # chunked reduce-scatter: core c's piece i holds the c-th 1/8 of
    # chunk i's row range
    out = np.empty((T, D), np.float32)
    for c in range(NCORES):
        oc = res.results[c]['out']
        o = 0
        for tt0, pp in ((0, 64), (4, 64), (8, 64), (12, 32), (14, 32)):
            g = tt0 * 128 + pp * c
            out[g:g + pp] = oc[o:o + pp]
            o += pp
    return out.reshape(B, T, D), res


def kernel(x, W_enc, W_enc_v, W_dec):
    out, _ = run(x, W_enc, W_enc_v, W_dec)
    return out.astype(np.float32)


# revision 32
# speedup vs baseline: 1.0350x; 1.0188x over previous
"""Trainium2 Bass kernel for nn_BDHLayer (sparse attention / BDH layer).

Sharding: 16 heads across 8 cores (2 heads per core, tensor parallel).
Decoder partial sums are combined with an on-chip ReduceScatter (bf16);
each core then applies the final layernorm+residual+rmsnorm to its T/8
slice.

All matmuls run in bf16 (fp32 PSUM accumulation). Host pre-transposes
weights/activations so every contraction dim lands on SBUF partitions.
The middle layernorm is applied as a post-GEMM correction:
  sqrelu(LN(yKV) @ Wv^T) = relu(z - Wsum*mu)^2 * r^2,
  z = yKV @ Wv^T, Wsum = sum_d Wv, r^2 = 1/(var+eps).

v2 restructure vs v1:
- enc loop tb-outer with per-(chunk, tb) segmented RoPE so scores start
  right after enc (no serial rope tail).
- yKV accumulation is column-trimmed on the causal diagonal like scores.
- LN stat rows broadcast via gpsimd partition_broadcast (PE never waits).
- Gating g = relu(z - Wsum*mu)^2 split across Act (relu, square) and DVE
  (v, xyw) so neither stalls the z matmuls.
- h1 gating runs tb-outer and the decoder chunk + ReduceScatter + final
  norms for each t-block are interleaved right behind it, hiding the
  collective latency behind remaining tensor work.
- Weight/activation DMAs are split and ordered so the first dependent
  matmul can start within a few us of each phase boundary.
"""

import sys

sys.path.insert(0, '/opt/trn_rl_repo')

import numpy as np
import ml_dtypes

import concourse.bass as bass
import concourse.bacc as bacc
import concourse.mybir as mybir
from concourse import tile
from concourse import bass_utils
from concourse import bass_isa

BF = ml_dtypes.bfloat16
FP32 = np.float32

B, T, D = 1, 2048, 1024
NH = 16
N = 1024            # neurons per head
CS = 256            # rotary chunk size
BASE = 2.0 ** 16
SCALE_BASE = 512.0
LN_EPS = 1e-5
RMS_EPS = 1e-6

NCORES = 8
HPC = NH // NCORES  # heads per core = 2
TS = T // NCORES    # output rows per core = 256

NT = N // 128       # 8 n-tiles per head
DT = D // 128       # 8 d-tiles
TT = T // 128       # 16 t-tiles
TB = T // 512       # 4 t-blocks
DB = D // 512       # 2 d-blocks

dt = mybir.dt
Alu = mybir.AluOpType
Act = mybir.ActivationFunctionType


# ---------------------------------------------------------------- host tables

def _rope_tables():
    idx = np.arange(0, CS, 2, dtype=np.float64)
    inv_freq = 1.0 / (BASE ** (idx / CS))
    t = np.arange(T, dtype=np.float64)
    freqs = t[:, None] * inv_freq[None, :]              # (T, 128)
    scale_vec = (idx + 0.4 * CS) / (1.4 * CS)
    power = (t - T // 2) / SCALE_BASE
    scale = scale_vec[None, :] ** power[:, None]        # (T, 128)
    cos = (np.cos(freqs) * scale).astype(np.float32)
    sin = (np.sin(freqs) * scale).astype(np.float32)
    # transpose to (128, T): row = pair index within chunk, col = t
    return np.ascontiguousarray(cos.T), np.ascontiguousarray(sin.T)


def _masks():
    # scoresT tile layout: [u_p (128), t_f (512)]; diagonal block j keeps
    # strictly-causal u < t, i.e. 128*j + u_p < t_f.
    m = np.zeros((4, 128, 512), dtype=np.float32)
    up = np.arange(128)[:, None]
    tf = np.arange(512)[None, :]
    for j in range(4):
        m[j] = (128 * j + up < tf).astype(np.float32)
    return m


# ------------------------------------------------------------------- builder

def _emit(nc, tc, tens):
    x_bf, xT_bf, xs_f32 = tens['x_bf'], tens['xT_bf'], tens['xs_f32']
    wencT, wencvT, wdecT = tens['wencT'], tens['wencvT'], tens['wdecT']
    wsumT, cosT_d, sinT_d, masks_d = (tens['wsumT'], tens['cosT'],
                                      tens['sinT'], tens['masks'])
    out_d, xy_d = tens['out'], tens['xy_d']
    bounce_in, bounce_out = tens['bounce_in'], tens['bounce_out']

    f32, bf16 = dt.float32, dt.bfloat16

    from contextlib import ExitStack
    with ExitStack() as ctx:
        p_const = ctx.enter_context(
            tc.tile_pool(name="const", bufs=1, side="right"))
        p_psum = ctx.enter_context(
            tc.tile_pool(name="psum", bufs=6, space="PSUM"))
        p_psum_v = ctx.enter_context(
            tc.tile_pool(name="psumv", bufs=2, space="PSUM"))

        # ---- constants; their DMAs are issued inside h0's enc scope so the
        # startup HBM bandwidth goes to the first-needed weights first
        cos_sb = p_const.tile([128, T], bf16, tag="cos")
        sin_sb = p_const.tile([128, T], bf16, tag="sin")
        mask_sb = p_const.tile([128, 4 * 512], bf16, tag="masks")
        wsum_sb = p_const.tile([128, HPC * NT], f32, tag="wsum")
        ones_bf = p_const.tile([128, 1], bf16, tag="ones_bf")
        nc.vector.memset(ones_bf[:], 1.0)

        for h in range(HPC):
            with ExitStack() as hctx:
                p_head = hctx.enter_context(
                    tc.tile_pool(name=f"head{h}", bufs=1, side="right"))
                qsq = p_head.tile([128, NT * T], bf16, tag="qsq")
                ykv = p_head.tile([128, DT * T], bf16, tag="ykv")
                mu_b = p_head.tile([128, T], bf16, tag="mu_b")
                r2_b = p_head.tile([128, T], bf16, tag="r2_b")
                p_wv = hctx.enter_context(
                    tc.tile_pool(name=f"wv{h}", bufs=1, side="left"))
                wv_sb = p_wv.tile([128, DT * N], bf16, tag="wv")

                with ExitStack() as mctx:
                    p_mid = mctx.enter_context(
                        tc.tile_pool(name=f"mid{h}", bufs=1, side="right"))
                    qtr = p_mid.tile([128, NT * T], bf16, tag="qtr")
                    x_sb = p_mid.tile([128, TT * D], bf16, tag="x")

                    # =========================== ENC + ROPE ================
                    with ExitStack() as ectx:
                        p_enc = ectx.enter_context(
                            tc.tile_pool(name=f"enc{h}", bufs=1))
                        p_scr = ectx.enter_context(
                            tc.tile_pool(name=f"escr{h}", bufs=3))
                        p_rt = ectx.enter_context(
                            tc.tile_pool(name=f"rt{h}", bufs=4))

                        p_xf = ectx.enter_context(
                            tc.tile_pool(name=f"xf{h}", bufs=2))
                        wenc_sb = p_enc.tile([128, DT * N], bf16, tag="wenc")
                        # first-needed first: wenc cols 0:512 + xfull tb0,
                        # then the rest; x_sb (for yKV) trails behind.
                        # h0 startup splits across sync+gpsimd; h1's loads
                        # stay off gpsimd (it is busy storing h0's xy).
                        eng2 = nc.gpsimd if h == 0 else nc.sync

                        def load_xf(tb):
                            t = p_xf.tile([128, DT * 512], bf16, tag="xf")
                            for dtt in range(DT):
                                eng = eng2 if (tb + dtt) % 2 == 0 else nc.sync
                                eng.dma_start(
                                    t[:, dtt * 512:(dtt + 1) * 512],
                                    xT_bf[dtt * 128:(dtt + 1) * 128,
                                          tb * 512:(tb + 1) * 512])
                            return t

                        for dtt in range(DT):
                            nc.sync.dma_start(
                                wenc_sb[:, dtt * N:dtt * N + 512],
                                wencT[h, dtt * 128:(dtt + 1) * 128, 0:512])
                        if h == 0:
                            xf_t = p_xf.tile([128, DT * 512], bf16, tag="xf")
                            for dtt in range(DT):
                                nc.gpsimd.dma_start(
                                    xf_t[:, dtt * 512:(dtt + 1) * 512],
                                    xT_bf[dtt * 128:(dtt + 1) * 128, 0:512])
                        else:
                            xf_t = load_xf(0)
                        for dtt in range(DT):
                            nc.sync.dma_start(
                                wenc_sb[:, dtt * N + 512:(dtt + 1) * N],
                                wencT[h, dtt * 128:(dtt + 1) * 128, 512:N])
                        if h == 0:
                            # constants ride behind the critical enc loads
                            nc.gpsimd.dma_start(cos_sb[:], cosT_d[:])
                            nc.gpsimd.dma_start(sin_sb[:], sinT_d[:])
                        xf_next = load_xf(1)
                        for tt in range(TT):
                            eng = nc.sync if tt % 2 == 0 else eng2
                            eng.dma_start(x_sb[:, tt * D:(tt + 1) * D],
                                          x_bf[tt * 128:(tt + 1) * 128, :])
                        if h == 0:
                            for j in range(4):
                                nc.gpsimd.dma_start(
                                    mask_sb[:, j * 512:(j + 1) * 512],
                                    masks_d[j, :, :])
                            for hh in range(HPC):
                                nc.gpsimd.dma_start(
                                    wsum_sb[:, hh * NT:(hh + 1) * NT],
                                    wsumT[hh, :, :])

                        for tb in range(TB):
                            tsl = slice(tb * 512, (tb + 1) * 512)
                            xfull = xf_t
                            for nt in range(NT):
                                ps = p_psum.tile([128, 512], f32, tag="mm")
                                for dtt in range(DT):
                                    nc.tensor.matmul(
                                        ps[:],
                                        wenc_sb[:, dtt * N + nt * 128:
                                                dtt * N + nt * 128 + 128],
                                        xfull[:, dtt * 512:(dtt + 1) * 512],
                                        start=(dtt == 0), stop=(dtt == DT - 1))
                                relu_t = p_scr.tile([128, 512], f32,
                                                    tag="relu")
                                nc.scalar.activation(relu_t[:], ps[:],
                                                     Act.Relu)
                                nc.vector.tensor_mul(
                                    qsq[:, nt * T + tb * 512:
                                        nt * T + tb * 512 + 512],
                                    relu_t[:], relu_t[:])
                                if nt % 2 == 1:
                                    # rope this (chunk pair, tb) segment now
                                    c = nt // 2
                                    a = qsq[:, (2 * c) * T + tb * 512:
                                            (2 * c) * T + (tb + 1) * 512]
                                    b = qsq[:, (2 * c + 1) * T + tb * 512:
                                            (2 * c + 1) * T + (tb + 1) * 512]
                                    t1 = p_rt.tile([128, 512], bf16, tag="rt")
                                    t2 = p_rt.tile([128, 512], bf16, tag="rt")
                                    nc.vector.tensor_mul(t1[:], a,
                                                         cos_sb[:, tsl])
                                    nc.vector.tensor_mul(t2[:], b,
                                                         sin_sb[:, tsl])
                                    nc.vector.tensor_sub(
                                        qtr[:, (2 * c) * T + tb * 512:
                                            (2 * c) * T + (tb + 1) * 512],
                                        t1[:], t2[:])
                                    t3 = p_rt.tile([128, 512], bf16, tag="rt")
                                    t4 = p_rt.tile([128, 512], bf16, tag="rt")
                                    nc.vector.tensor_mul(t3[:], b,
                                                         cos_sb[:, tsl])
                                    nc.vector.tensor_mul(t4[:], a,
                                                         sin_sb[:, tsl])
                                    nc.vector.tensor_add(
                                        qtr[:, (2 * c + 1) * T + tb * 512:
                                            (2 * c + 1) * T + (tb + 1) * 512],
                                        t3[:], t4[:])
                            xf_t = xf_next
                            if tb < TB - 2:
                                xf_next = load_xf(tb + 2)

                    # ======================= SCORES + yKV ===================
                    with ExitStack() as sctx:
                        p_sct = sctx.enter_context(
                            tc.tile_pool(name=f"sct{h}", bufs=2))
                        p_sq = sctx.enter_context(
                            tc.tile_pool(name=f"sq{h}", bufs=1))
                        p_row = sctx.enter_context(
                            tc.tile_pool(name=f"row{h}", bufs=1))

                        # wv lands during the scores phase so the gating z
                        # matmuls never wait on it; gpsimd queue so it can't
                        # delay the next head's enc loads on sync
                        for dtt in range(DT):
                            nc.gpsimd.dma_start(
                                wv_sb[:, dtt * N:(dtt + 1) * N],
                                wencvT[h, dtt * 128:(dtt + 1) * 128, :])

                        for tb in range(TB):
                            ub_max = 4 * tb + 4
                            sct = p_sct.tile([128, 16 * 512], bf16, tag="sct")
                            for ub in range(ub_max):
                                j = ub - 4 * tb
                                off = 128 * j if j > 0 else 0
                                w = 512 - off
                                ps = p_psum.tile([128, 512], f32, tag="mm")
                                for nt in range(NT):
                                    nc.tensor.matmul(
                                        ps[:, :w],
                                        qtr[:, nt * T + ub * 128:
                                            nt * T + ub * 128 + 128],
                                        qtr[:, nt * T + tb * 512 + off:
                                            nt * T + (tb + 1) * 512],
                                        start=(nt == 0), stop=(nt == NT - 1))
                                base = ub * 512
                                if j >= 0:
                                    nc.vector.tensor_mul(
                                        sct[:, base + off:base + 512],
                                        ps[:, :w],
                                        mask_sb[:, j * 512 + off:
                                                (j + 1) * 512])
                                else:
                                    nc.scalar.copy(sct[:, base:base + 512],
                                                   ps[:])

                            sq_half = p_sq.tile([128, 4 * 512], bf16,
                                                tag="sq")
                            ssq_ps = p_psum_v.tile([1, 512], f32, tag="st")
                            for dtt in range(DT):
                                ps2 = p_psum.tile([128, 512], f32, tag="mm")
                                for ub in range(ub_max):
                                    j = ub - 4 * tb
                                    off = 128 * j if j > 0 else 0
                                    nc.tensor.matmul(
                                        ps2[:, off:],
                                        x_sb[:, ub * D + dtt * 128:
                                             ub * D + dtt * 128 + 128],
                                        sct[:, ub * 512 + off:
                                            (ub + 1) * 512],
                                        start=(ub == 0),
                                        stop=(ub == ub_max - 1))
                                nc.scalar.copy(
                                    ykv[:, dtt * T + tb * 512:
                                        dtt * T + tb * 512 + 512], ps2[:])
                                nc.scalar.square(
                                    sq_half[:, (dtt % 4) * 512:
                                            (dtt % 4 + 1) * 512],
                                    ps2[:])
                                if dtt % 4 == 3:
                                    for q4 in range(4):
                                        nc.tensor.matmul(
                                            ssq_ps[:], ones_bf[:],
                                            sq_half[:, q4 * 512:
                                                    (q4 + 1) * 512],
                                            start=(dtt == 3 and q4 == 0),
                                            stop=(dtt == DT - 1 and q4 == 3))

                            mean_ps = p_psum_v.tile([1, 512], f32, tag="st")
                            for dtt in range(DT):
                                nc.tensor.matmul(
                                    mean_ps[:], ones_bf[:],
                                    ykv[:, dtt * T + tb * 512:
                                        dtt * T + tb * 512 + 512],
                                    start=(dtt == 0), stop=(dtt == DT - 1))
                            sl = slice(tb * 512, (tb + 1) * 512)
                            mu_r = p_row.tile([1, 512], bf16, tag="mu_r")
                            ssq_r = p_row.tile([1, 512], f32, tag="ssq_r")
                            musq_r = p_row.tile([1, 512], bf16, tag="musq_r")
                            r2_r = p_row.tile([1, 512], bf16, tag="r2_r")
                            nc.scalar.mul(mu_r[:], mean_ps[:], 1.0 / D)
                            nc.scalar.mul(ssq_r[:], ssq_ps[:], 1.0 / D)
                            nc.vector.tensor_mul(musq_r[:], mu_r[:], mu_r[:])
                            nc.vector.tensor_sub(ssq_r[:], ssq_r[:],
                                                 musq_r[:])
                            nc.vector.tensor_scalar_add(
                                ssq_r[:], ssq_r[:], LN_EPS)
                            nc.vector.reciprocal(ssq_r[:], ssq_r[:])
                            nc.vector.tensor_copy(r2_r[:], ssq_r[:])
                            # broadcast stat rows to all partitions (gpsimd,
                            # keeps the PE out of the dependency chain)
                            nc.gpsimd.partition_broadcast(
                                mu_b[:, sl], mu_r[:], channels=128)
                            nc.gpsimd.partition_broadcast(
                                r2_b[:, sl], r2_r[:], channels=128)

                # ================== Z / GATING (+ DECODER on h1) ===========
                with ExitStack() as gctx:
                    p_zs = gctx.enter_context(
                        tc.tile_pool(name=f"zs{h}", bufs=3, side="left"))
                    p_zq = gctx.enter_context(
                        tc.tile_pool(name=f"zq{h}", bufs=3, side="left"))

                    if h == 0:
                        p_xyw = gctx.enter_context(
                            tc.tile_pool(name="xyw0", bufs=3, side="left"))
                    else:
                        p_wd = gctx.enter_context(
                            tc.tile_pool(name="wd", bufs=1, side="left"))
                        p_xy1 = gctx.enter_context(
                            tc.tile_pool(name="xy1", bufs=2, side="left"))
                        p_xy0 = gctx.enter_context(
                            tc.tile_pool(name="xy0", bufs=2, side="left"))
                        p_ym = gctx.enter_context(
                            tc.tile_pool(name="ym", bufs=3, side="left"))
                        p_fin = gctx.enter_context(
                            tc.tile_pool(name="fin", bufs=1, side="left"))
                        wd_sb = p_wd.tile([128, HPC * NT * D], bf16, tag="wd")
                        for r in range(HPC * NT):
                            eng = nc.sync if r % 2 == 1 else nc.gpsimd
                            eng.dma_start(wd_sb[:, r * D:(r + 1) * D],
                                          wdecT[r * 128:(r + 1) * 128, :])

                        def fetch_xy0(tb):
                            t = p_xy0.tile([128, NT * 512], bf16, tag="xy0")
                            for nt in range(NT):
                                nc.scalar.dma_start(
                                    t[:, nt * 512:(nt + 1) * 512],
                                    xy_d[nt, :, tb * 512:(tb + 1) * 512])
                            return t
                        xy0_t = fetch_xy0(0)
                        xy0_next = fetch_xy0(1)
                        pending_norms = []

                        def emit_norms(PO, P):
                            yt = p_fin.tile([P, D], bf16, tag="yt")
                            nc.sync.dma_start(yt[:],
                                              bounce_out[PO:PO + P, :])
                            xt = p_fin.tile([P, D], f32, tag="xt")
                            nc.sync.dma_start(xt[:], xs_f32[PO:PO + P, :])

                            mu_c = p_fin.tile([P, 1], f32, tag="mu_c")
                            nc.vector.tensor_reduce(mu_c[:], yt[:],
                                                    mybir.AxisListType.X,
                                                    Alu.add)
                            nc.scalar.mul(mu_c[:], mu_c[:], 1.0 / D)
                            sq_t = p_fin.tile([P, D], f32, tag="sq_t")
                            ssq_c = p_fin.tile([P, 1], f32, tag="ssq_c")
                            nc.vector.scalar_tensor_tensor(
                                sq_t[:], yt[:], 1.0, yt[:],
                                op0=Alu.mult, op1=Alu.mult,
                                accum_out=ssq_c[:])
                            nc.scalar.mul(ssq_c[:], ssq_c[:], 1.0 / D)
                            musq_c = p_fin.tile([P, 1], f32, tag="musq_c")
                            nc.vector.tensor_mul(musq_c[:], mu_c[:],
                                                 mu_c[:])
                            nc.vector.tensor_sub(ssq_c[:], ssq_c[:],
                                                 musq_c[:])
                            nc.vector.tensor_scalar_add(ssq_c[:], ssq_c[:],
                                                        LN_EPS)
                            r_c = p_fin.tile([P, 1], f32, tag="r_c")
                            nc.vector.reciprocal(r_c[:], ssq_c[:])
                            nc.scalar.sqrt(r_c[:], r_c[:])

                            zt = p_fin.tile([P, D], f32, tag="zt")
                            nc.vector.tensor_scalar(zt[:], yt[:],
                                                    mu_c[:], r_c[:],
                                                    op0=Alu.subtract,
                                                    op1=Alu.mult)
                            nc.vector.tensor_add(zt[:], zt[:], xt[:])

                            rr_c = p_fin.tile([P, 1], f32, tag="rr_c")
                            nc.vector.scalar_tensor_tensor(
                                sq_t[:], zt[:], 1.0, zt[:],
                                op0=Alu.mult, op1=Alu.mult,
                                accum_out=rr_c[:])
                            nc.scalar.mul(rr_c[:], rr_c[:], 1.0 / D)
                            nc.vector.tensor_scalar_add(rr_c[:], rr_c[:],
                                                        RMS_EPS)
                            nc.vector.reciprocal(rr_c[:], rr_c[:])
                            nc.scalar.sqrt(rr_c[:], rr_c[:])

                            ot = p_fin.tile([P, D], f32, tag="ot")
                            nc.vector.tensor_scalar_mul(ot[:], zt[:],
                                                        rr_c[:])
                            nc.sync.dma_start(out_d[PO:PO + P, :], ot[:])

                    for tb in range(TB):
                        sl = slice(tb * 512, (tb + 1) * 512)
                        if h == 1:
                            xy1 = p_xy1.tile([128, NT * 512], bf16, tag="xy1")
                        for nt in range(NT):
                            q_t = p_zq.tile([128, 512], bf16, tag="q")
                            nc.vector.tensor_mul(
                                q_t[:], qsq[:, nt * T + tb * 512:
                                            nt * T + (tb + 1) * 512],
                                r2_b[:, sl])
                            ps3 = p_psum.tile([128, 512], f32, tag="mm")
                            for dtt in range(DT):
                                nc.tensor.matmul(
                                    ps3[:],
                                    wv_sb[:, dtt * N + nt * 128:
                                          dtt * N + nt * 128 + 128],
                                    ykv[:, dtt * T + tb * 512:
                                        dtt * T + tb * 512 + 512],
                                    start=(dtt == 0), stop=(dtt == DT - 1))
                            # v = Wsum[n]*mu[t] - z ; g = relu(-v)^2
                            v_t = p_zs.tile([128, 512], f32, tag="v")
                            nc.vector.scalar_tensor_tensor(
                                v_t[:], mu_b[:, sl],
                                wsum_sb[:, h * NT + nt:h * NT + nt + 1],
                                ps3[:], op0=Alu.mult, op1=Alu.subtract)
                            t1_t = p_zs.tile([128, 512], bf16, tag="t1")
                            nc.scalar.activation(t1_t[:], v_t[:], Act.Relu,
                                                 scale=-1.0)
                            g_t = p_zs.tile([128, 512], bf16, tag="g")
                            nc.scalar.square(g_t[:], t1_t[:])
                            if h == 0:
                                xyw = p_xyw.tile([128, 512], bf16, tag="xyw")
                                nc.vector.tensor_mul(xyw[:], g_t[:], q_t[:])
                                nc.gpsimd.dma_start(
                                    xy_d[nt, :, tb * 512:(tb + 1) * 512],
                                    xyw[:])
                            else:
                                nc.vector.tensor_mul(
                                    xy1[:, nt * 512:(nt + 1) * 512],
                                    g_t[:], q_t[:])

                        if h == 1:
                            # -------- decoder + RS + final norms; last
                            # t-block splits in two so the tail collective
                            # is half-size
                            subs = ([(4 * tb, 4)] if tb < TB - 1
                                    else [(12, 2), (14, 2)])
                            for tt0, ntt in subs:
                                for tt in range(tt0, tt0 + ntt):
                                    to = (tt - 4 * tb) * 128
                                    for db in range(DB):
                                        ps4 = p_psum.tile([128, 512], f32,
                                                          tag="mm")
                                        idx = 0
                                        for hh in range(HPC):
                                            src = xy0_t if hh == 0 else xy1
                                            for nt in range(NT):
                                                nc.tensor.matmul(
                                                    ps4[:],
                                                    src[:, nt * 512 + to:
                                                        nt * 512 + to + 128],
                                                    wd_sb[:,
                                                          (hh * NT + nt) * D +
                                                          db * 512:
                                                          (hh * NT + nt) * D +
                                                          db * 512 + 512],
                                                    start=(idx == 0),
                                                    stop=(idx ==
                                                          HPC * NT - 1))
                                                idx += 1
                                        ym_t = p_ym.tile([128, 512], bf16,
                                                         tag="ym")
                                        nc.scalar.copy(ym_t[:], ps4[:])
                                        nc.sync.dma_start(
                                            bounce_in[tt * 128:(tt + 1) * 128,
                                                      db * 512:
                                                      (db + 1) * 512],
                                            ym_t[:])
                                r0 = tt0 * 128
                                rows = ntt * 128
                                o0 = r0 // NCORES
                                P = rows // NCORES
                                nc.gpsimd.collective_compute(
                                    "ReduceScatter", Alu.add,
                                    replica_groups=[list(range(NCORES))],
                                    ins=[bounce_in[r0:r0 + rows, :].opt()],
                                    outs=[bounce_out[o0:o0 + P, :].opt()])
                                # norms are deferred one chunk so the next
                                # gating/decode ops sit ahead of them in the
                                # Act/DVE/sync queues (no head-of-line block
                                # behind the collective)
                                pending_norms.append((o0, P))
                                if len(pending_norms) > 1:
                                    emit_norms(*pending_norms.pop(0))
                            xy0_t = xy0_next
                            if tb < TB - 2:
                                xy0_next = fetch_xy0(tb + 2)
                    if h == 1:
                        while pending_norms:
                            emit_norms(*pending_norms.pop(0))


def build(debug=False):
    nc = bacc.Bacc("TRN2", target_bir_lowering=False, debug=False,
                   num_devices=NCORES)
    f32, bf16 = dt.float32, dt.bfloat16
    tens = {
        'x_bf': nc.dram_tensor("x_bf", [T, D], bf16, kind="ExternalInput"),
        'xT_bf': nc.dram_tensor("xT_bf", [D, T], bf16, kind="ExternalInput"),
        'xs_f32': nc.dram_tensor("xs_f32", [TS, D], f32,
                                 kind="ExternalInput"),
        'wencT': nc.dram_tensor("wencT", [HPC, D, N], bf16,
                                kind="ExternalInput"),
        'wencvT': nc.dram_tensor("wencvT", [HPC, D, N], bf16,
                                 kind="ExternalInput"),
        'wdecT': nc.dram_tensor("wdecT", [HPC * N, D], bf16,
                                kind="ExternalInput"),
        'wsumT': nc.dram_tensor("wsumT", [HPC, 128, NT], f32,
                                kind="ExternalInput"),
        'cosT': nc.dram_tensor("cosT", [128, T], bf16, kind="ExternalInput"),
        'sinT': nc.dram_tensor("sinT", [128, T], bf16, kind="ExternalInput"),
        'masks': nc.dram_tensor("masks", [4, 128, 512], bf16,
                                kind="ExternalInput"),
        'out': nc.dram_tensor("out", [TS, D], f32, kind="ExternalOutput"),
        'xy_d': nc.dram_tensor("xy_d", [NT, 128, T], bf16, kind="Internal"),
        'bounce_in': nc.dram_tensor("bounce_in", [T, D], bf16,
                                    kind="Internal"),
        'bounce_out': nc.dram_tensor("bounce_out", [TS, D], bf16,
                                     kind="Internal"),
    }

    with tile.TileContext(nc) as tc:
        _emit(nc, tc, tens)
    nc.compile()
    return nc


def make_in_maps(x, W_enc, W_enc_v, W_dec):
    x2 = np.asarray(x, FP32).reshape(T, D)
    x_bf = x2.astype(BF)
    xT_bf = np.ascontiguousarray(x2.T).astype(BF)
    cosT, sinT = _rope_tables()
    cosT, sinT = cosT.astype(BF), sinT.astype(BF)
    masks = _masks().astype(BF)
    wsum = np.asarray(W_enc_v, FP32).sum(axis=2)          # (NH, N)

    in_maps = []
    for k in range(NCORES):
        h0 = HPC * k
        wencT = np.ascontiguousarray(
            np.asarray(W_enc[h0:h0 + HPC], FP32).transpose(0, 2, 1)
        ).astype(BF)
        wencvT = np.ascontiguousarray(
            np.asarray(W_enc_v[h0:h0 + HPC], FP32).transpose(0, 2, 1)
        ).astype(BF)
        wdecT = np.ascontiguousarray(
            np.asarray(W_dec[:, h0 * N:(h0 + HPC) * N], FP32).T
        ).astype(BF)
        wsumT = np.ascontiguousarray(
            wsum[h0:h0 + HPC].reshape(HPC, NT, 128).transpose(0, 2, 1))
        in_maps.append({
            'x_bf': x_bf,
            'xT_bf': xT_bf,
            'xs_f32': np.ascontiguousarray(np.concatenate(
                [x2[tt0 * 128 + pp * k:tt0 * 128 + pp * k + pp]
                 for tt0, pp in ((0, 64), (4, 64), (8, 64),
                                 (12, 32), (14, 32))], axis=0)),
            'wencT': wencT,
            'wencvT': wencvT,
            'wdecT': wdecT,
            'wsumT': wsumT,
            'cosT': cosT,
            'sinT': sinT,
            'masks': masks,
        })
    return in_maps


_nc_cache = {}


def get_nc(debug=False):
    if debug not in _nc_cache:
        _nc_cache[debug] = build(debug=debug)
    return _nc_cache[debug]


def run(x, W_enc, W_enc_v, W_dec, debug=False, trace=False):
    nc = get_nc(debug=debug)
    in_maps = make_in_maps(x, W_enc, W_enc_v, W_dec)
    res = bass_utils.run_bass_kernel_spmd(
        nc, in_maps, core_ids=list(range(NCORES)), trace=trace)
    # chunked reduce-scatter: core c's piece i holds the c-th 1/8 of
    # chunk i's row range
    out = np.empty((T, D), np.float32)
    for c in range(NCORES):
        oc = res.results[c]['out']
        o = 0
        for tt0, pp in ((0, 64), (4, 64), (8, 64), (12, 32), (14, 32)):
            g = tt0 * 128 + pp * c
            out[g:g + pp] = oc[o:o + pp]
            o += pp
    return out.reshape(B, T, D), res


def kernel(x, W_enc, W_enc_v, W_dec):
    out, _ = run(x, W_enc, W_enc_v, W_dec)
    return out.astype(np.float32)
